# revision 25
# baseline (speedup 1.0000x reference)
"""MCR2 loss kernel for 8 Trainium2 NeuronCores.

Class-sorted data-parallel sharding: the host permutes samples so each
class occupies a contiguous, zero-padded block of CAP rows (one-hot
masking then costs nothing on device).  Each core streams its 76800-row
shard of the permuted Z once as fp8 (e4m3: the quantization bias
cancels between loss_R and loss_Rc, measured 3.1e-3 relative error).  Tensor work is batched four
128-sample tiles per matmul: stationary = moving = a [128, 128] column
block of four adjacent tiles, accumulated into one [128, 128] PSUM
region per 15360-row chunk.  The four diagonal 32x32 blocks of each
region are the per-tile Grams (off-diagonal cross blocks accumulate
into distinct PSUM addresses and are never read).  Every chunk lies
inside one class block, so chunk-Grams sum to class-Grams on the host,
where the 32x32 logdets are evaluated in float64.
"""

import sys

sys.path.insert(0, "/opt/trn_rl_repo")

import numpy as np

import concourse.bacc as bacc
import concourse.bass as bass  # noqa: F401  (kept for parity with bacc deps)
import concourse.mybir as mybir
import concourse.tile as tile
from concourse.bass_utils import run_bass_kernel_spmd

N, D, C = 600000, 32, 10
EPS = 0.5
NCORES = 8
CAP = 61440                      # padded rows per class block (~60000 + 6 sigma)
ROWS_PER_CORE = C * CAP // NCORES        # 76800
CHUNK_TILES = 120                # 128-sample matmul tiles per chunk
CHUNK_ROWS = 128 * CHUNK_TILES           # 15360
CHUNKS_PER_CORE = ROWS_PER_CORE // CHUNK_ROWS    # 5
CHUNKS_PER_CLASS = CAP // CHUNK_ROWS             # 4
GROUPS = CHUNK_TILES // 4        # 30 four-tile matmul groups per chunk
WARMUPS = 22                     # dummy matmuls that ramp the PE clock

_cache = {}


def _build_program():
    nc = bacc.Bacc(None)
    f8 = mybir.dt.float8e4
    f32 = mybir.dt.float32
    z_dram = nc.dram_tensor("Z", [ROWS_PER_CORE, D], f8, kind="ExternalInput")
    out_dram = nc.dram_tensor(
        "grams", [128, CHUNKS_PER_CORE * D], f32, kind="ExternalOutput"
    )

    with tile.TileContext(nc) as tc:
        with (
            tc.tile_pool(name="z", bufs=CHUNKS_PER_CORE) as z_pool,
            tc.tile_pool(name="outp", bufs=1) as out_pool,
            tc.tile_pool(name="psum", bufs=CHUNKS_PER_CORE, space="PSUM") as psum_pool,
            tc.tile_pool(name="warm", bufs=1) as warm_pool,
            tc.tile_pool(name="wpsum", bufs=1, space="PSUM") as wpsum_pool,
        ):
            zv = z_dram.rearrange("(c p t) d -> c p (t d)", p=128, t=CHUNK_TILES)

            # All chunk DMAs are issued up front, every chunk halved across
            # the two hardware DGE queues (Sync and Scalar engines) so both
            # pull from HBM at once and halves land every ~1.2us, keeping
            # the PE fed; the whole shard stays resident in SBUF.
            # The 16 DMA rings wake staggered; one descriptor per ring (a
            # 16-partition sliver on each queue) gets them all spinning
            # while the framework preamble still runs.
            ring_sb = warm_pool.tile([16, 4], f8)
            nc.sync.dma_start(ring_sb[:, 0:2], zv[0][0:16, 0:2])
            nc.scalar.dma_start(ring_sb[:, 2:4], zv[0][0:16, 2:4])

            half = CHUNK_TILES // 2 * D
            quarter = CHUNK_TILES // 4 * D
            z_tiles = []
            for c in range(CHUNKS_PER_CORE):
                z_sb = z_pool.tile([128, CHUNK_TILES * D], f8, tag="z")
                if c == CHUNKS_PER_CORE - 1:
                    # The last chunk lands in four fine-grained pieces so the
                    # PE drains it almost as soon as the final bytes arrive.
                    # The scalar queue runs ~1.5us behind sync, so sync gets
                    # the later-consumed pieces.
                    for q in range(4):
                        lo, hi = q * quarter, (q + 1) * quarter
                        eng = nc.scalar if q % 2 == 0 else nc.sync
                        eng.dma_start(z_sb[:, lo:hi], zv[c][:, lo:hi])
                elif c == CHUNKS_PER_CORE - 2:
                    nc.scalar.dma_start(z_sb[:, :half], zv[c][:, :half])
                    nc.sync.dma_start(z_sb[:, half:], zv[c][:, half:])
                else:
                    nc.sync.dma_start(z_sb[:, :half], zv[c][:, :half])
                    nc.scalar.dma_start(z_sb[:, half:], zv[c][:, half:])
                z_tiles.append(z_sb)

            # The PE clock ramps (0.65 -> 1.2 -> 2.4 GHz) only after a few
            # microseconds of continuous work.  Dummy matmuls on a zeroed
            # scratch tile burn the otherwise-idle DMA-ramp window so the
            # real matmuls start at full clock.
            warm_z = warm_pool.tile([128, 128], f8)
            nc.vector.memset(warm_z[:], 0.0)
            wacc = wpsum_pool.tile([128, 128], f32)
            for _ in range(WARMUPS):
                nc.tensor.matmul(wacc[:], warm_z[:], warm_z[:], start=True, stop=True)

            # Each chunk accumulates into its own PSUM bank, so a fresh
            # chunk's start=True never waits on an earlier extraction.  As
            # soon as a chunk's last matmul retires its four diagonal 32x32
            # blocks are copied to SBUF, overlapping later chunks' matmuls;
            # late chunks split the copies across DVE and Activation to
            # shorten the tail.
            out_sb = out_pool.tile([128, CHUNKS_PER_CORE * D], f32)
            for c in range(CHUNKS_PER_CORE):
                z_sb = z_tiles[c]
                acc = psum_pool.tile([128, 128], f32, tag="acc")
                for g in range(GROUPS):
                    zg = z_sb[:, g * 4 * D : (g + 1) * 4 * D]
                    nc.tensor.matmul(
                        acc[:],
                        zg,
                        zg,
                        start=(g == 0),
                        stop=(g == GROUPS - 1),
                    )
                for b in range(4):
                    dst = out_sb[b * D : (b + 1) * D, c * D : (c + 1) * D]
                    src = acc[b * D : (b + 1) * D, b * D : (b + 1) * D]
                    if c == 3 and b >= 2:
                        nc.scalar.mul(dst, src, 1.0)
                    else:
                        nc.vector.tensor_copy(dst, src)
                if c == 2:
                    nc.sync.dma_start(out_dram[:, : 3 * D], out_sb[:, : 3 * D])
            nc.sync.dma_start(out_dram[:, 3 * D :], out_sb[:, 3 * D :])

    nc.compile()
    return nc


def kernel(Z: np.ndarray, labels: np.ndarray) -> np.ndarray:
    Z = np.asarray(Z, dtype=np.float32)
    labels = np.asarray(labels, dtype=np.int32)

    if "nc" not in _cache:
        _cache["nc"] = _build_program()
    nc = _cache["nc"]

    counts = np.bincount(labels, minlength=C)
    order = np.argsort(labels, kind="stable")

    Zp = np.zeros([C * CAP, D], mybir.dt.np(mybir.dt.float8e4))
    host_extra = np.zeros([C, D, D], np.float64)
    off = 0
    for j in range(C):
        cnt = int(counts[j])
        take = min(cnt, CAP)
        Zp[j * CAP : j * CAP + take] = Z[order[off : off + take]]
        if cnt > CAP:
            extra = Z[order[off + take : off + cnt]].astype(np.float64)
            host_extra[j] = extra.T @ extra
        off += cnt

    in_maps = [
        {"Z": Zp[k * ROWS_PER_CORE : (k + 1) * ROWS_PER_CORE]}
        for k in range(NCORES)
    ]

    res = run_bass_kernel_spmd(nc, in_maps, core_ids=list(range(NCORES)))
    _cache["last_results"] = res

    gj = host_extra.copy()
    for k, r in enumerate(res.results):
        # [128, 160] -> bands summed -> [32, 5, 32] per-chunk partials
        g = r["grams"].astype(np.float64).reshape(4, D, CHUNKS_PER_CORE, D).sum(axis=0)
        for c in range(CHUNKS_PER_CORE):
            gj[(k * CHUNKS_PER_CORE + c) // CHUNKS_PER_CLASS] += g[:, c, :]

    g_all = gj.sum(axis=0)
    tr_pi = counts.astype(np.float64)

    nf, df = float(N), float(D)
    eye = np.eye(D)
    loss_r = 0.5 * np.linalg.slogdet(eye + (df / (nf * EPS)) * g_all)[1]
    loss_rc = 0.0
    for j in range(C):
        ld = np.linalg.slogdet(eye + (df / (tr_pi[j] * EPS)) * gj[j])[1]
        loss_rc += (tr_pi[j] / (2.0 * nf)) * ld
    loss_obj = loss_r - loss_rc
    return np.asarray([-loss_obj, loss_r, loss_rc], dtype=np.float32)


# revision 27
# speedup vs baseline: 1.0087x; 1.0087x over previous
"""MCR2 loss kernel for 8 Trainium2 NeuronCores.

Class-sorted data-parallel sharding: the host permutes samples so each
class occupies a contiguous, zero-padded block of CAP rows (one-hot
masking then costs nothing on device).  Each core streams its 76800-row
shard of the permuted Z once as fp8 (e4m3: the quantization bias
cancels between loss_R and loss_Rc, measured 3.1e-3 relative error).  Tensor work is batched four
128-sample tiles per matmul: stationary = moving = a [128, 128] column
block of four adjacent tiles, accumulated into one [128, 128] PSUM
region per 15360-row chunk.  The four diagonal 32x32 blocks of each
region are the per-tile Grams (off-diagonal cross blocks accumulate
into distinct PSUM addresses and are never read).  Every chunk lies
inside one class block, so chunk-Grams sum to class-Grams on the host,
where the 32x32 logdets are evaluated in float64.
"""

import sys

sys.path.insert(0, "/opt/trn_rl_repo")

import numpy as np

import concourse.bacc as bacc
import concourse.bass as bass  # noqa: F401  (kept for parity with bacc deps)
import concourse.mybir as mybir
import concourse.tile as tile
from concourse.bass_utils import run_bass_kernel_spmd

N, D, C = 600000, 32, 10
EPS = 0.5
NCORES = 8
CAP = 61440                      # padded rows per class block (~60000 + 6 sigma)
ROWS_PER_CORE = C * CAP // NCORES        # 76800
CHUNK_TILES = 120                # 128-sample matmul tiles per chunk
CHUNK_ROWS = 128 * CHUNK_TILES           # 15360
CHUNKS_PER_CORE = ROWS_PER_CORE // CHUNK_ROWS    # 5
CHUNKS_PER_CLASS = CAP // CHUNK_ROWS             # 4
GROUPS = CHUNK_TILES // 8        # 15 eight-tile DoubleRow matmul groups per chunk
WARMUPS = 16                     # dummy matmuls that ramp the PE clock

_cache = {}


def _build_program():
    nc = bacc.Bacc(None)
    f8 = mybir.dt.float8e4
    f32 = mybir.dt.float32
    z_dram = nc.dram_tensor("Z", [ROWS_PER_CORE, D], f8, kind="ExternalInput")
    out_dram = nc.dram_tensor(
        "grams", [128, CHUNKS_PER_CORE * D], f32, kind="ExternalOutput"
    )

    with tile.TileContext(nc) as tc:
        with (
            tc.tile_pool(name="z", bufs=CHUNKS_PER_CORE) as z_pool,
            tc.tile_pool(name="outp", bufs=1) as out_pool,
            tc.tile_pool(name="psum", bufs=CHUNKS_PER_CORE, space="PSUM") as psum_pool,
            tc.tile_pool(name="warm", bufs=1) as warm_pool,
            tc.tile_pool(name="wpsum", bufs=1, space="PSUM") as wpsum_pool,
        ):
            zv = z_dram.rearrange("(c p t) d -> c p (t d)", p=128, t=CHUNK_TILES)

            # All chunk DMAs are issued up front, every chunk halved across
            # the two hardware DGE queues (Sync and Scalar engines) so both
            # pull from HBM at once and halves land every ~1.2us, keeping
            # the PE fed; the whole shard stays resident in SBUF.
            # The 16 DMA rings wake staggered; one descriptor per ring (a
            # 16-partition sliver on each queue) gets them all spinning
            # while the framework preamble still runs.
            ring_sb = warm_pool.tile([16, 4], f8)
            nc.sync.dma_start(ring_sb[:, 0:2], zv[0][0:16, 0:2])
            nc.scalar.dma_start(ring_sb[:, 2:4], zv[0][0:16, 2:4])

            half = CHUNK_TILES // 2 * D
            quarter = CHUNK_TILES // 4 * D
            z_tiles = []
            for c in range(CHUNKS_PER_CORE):
                z_sb = z_pool.tile([128, CHUNK_TILES * D], f8, tag="z")
                if c == CHUNKS_PER_CORE - 1:
                    # The last chunk lands in four fine-grained pieces so the
                    # PE drains it almost as soon as the final bytes arrive.
                    # The scalar queue runs ~1.5us behind sync, so sync gets
                    # the later-consumed pieces.
                    for q in range(4):
                        lo, hi = q * quarter, (q + 1) * quarter
                        eng = nc.scalar if q % 2 == 0 else nc.sync
                        eng.dma_start(z_sb[:, lo:hi], zv[c][:, lo:hi])
                elif c == CHUNKS_PER_CORE - 2:
                    nc.scalar.dma_start(z_sb[:, :half], zv[c][:, :half])
                    nc.sync.dma_start(z_sb[:, half:], zv[c][:, half:])
                else:
                    nc.sync.dma_start(z_sb[:, :half], zv[c][:, :half])
                    nc.scalar.dma_start(z_sb[:, half:], zv[c][:, half:])
                z_tiles.append(z_sb)

            # The PE clock ramps (0.65 -> 1.2 -> 2.4 GHz) only after a few
            # microseconds of continuous work.  Dummy matmuls on a zeroed
            # scratch tile burn the otherwise-idle DMA-ramp window so the
            # real matmuls start at full clock.
            warm_z = warm_pool.tile([128, 256], f8)
            nc.vector.memset(warm_z[:], 0.0)
            warm3 = warm_z[:].rearrange("p (k x) -> p k x", k=2)
            wacc = wpsum_pool.tile([128, 128], f32)
            for _ in range(WARMUPS):
                nc.tensor.matmul(
                    wacc[:], warm3, warm3, start=True, stop=True,
                    perf_mode=mybir.MatmulPerfMode.DoubleRow,
                )

            # Each chunk accumulates into its own PSUM bank, so a fresh
            # chunk's start=True never waits on an earlier extraction.  As
            # soon as a chunk's last matmul retires its four diagonal 32x32
            # blocks are copied to SBUF, overlapping later chunks' matmuls;
            # late chunks split the copies across DVE and Activation to
            # shorten the tail.
            out_sb = out_pool.tile([128, CHUNKS_PER_CORE * D], f32)
            for c in range(CHUNKS_PER_CORE):
                z_sb = z_tiles[c]
                acc = psum_pool.tile([128, 128], f32, tag="acc")
                for g in range(GROUPS):
                    zg = z_sb[:, g * 8 * D : (g + 1) * 8 * D].rearrange(
                        "p (k x) -> p k x", k=2
                    )
                    nc.tensor.matmul(
                        acc[:],
                        zg,
                        zg,
                        start=(g == 0),
                        stop=(g == GROUPS - 1),
                        perf_mode=mybir.MatmulPerfMode.DoubleRow,
                    )
                for b in range(4):
                    dst = out_sb[b * D : (b + 1) * D, c * D : (c + 1) * D]
                    src = acc[b * D : (b + 1) * D, b * D : (b + 1) * D]
                    if c == 3 and b >= 2:
                        nc.scalar.mul(dst, src, 1.0)
                    else:
                        nc.vector.tensor_copy(dst, src)
                if c == 2:
                    nc.sync.dma_start(out_dram[:, : 3 * D], out_sb[:, : 3 * D])
            nc.sync.dma_start(out_dram[:, 3 * D :], out_sb[:, 3 * D :])

    nc.compile()
    return nc


def kernel(Z: np.ndarray, labels: np.ndarray) -> np.ndarray:
    Z = np.asarray(Z, dtype=np.float32)
    labels = np.asarray(labels, dtype=np.int32)

    if "nc" not in _cache:
        _cache["nc"] = _build_program()
    nc = _cache["nc"]

    counts = np.bincount(labels, minlength=C)
    order = np.argsort(labels, kind="stable")

    Zp = np.zeros([C * CAP, D], mybir.dt.np(mybir.dt.float8e4))
    host_extra = np.zeros([C, D, D], np.float64)
    off = 0
    for j in range(C):
        cnt = int(counts[j])
        take = min(cnt, CAP)
        Zp[j * CAP : j * CAP + take] = Z[order[off : off + take]]
        if cnt > CAP:
            extra = Z[order[off + take : off + cnt]].astype(np.float64)
            host_extra[j] = extra.T @ extra
        off += cnt

    in_maps = [
        {"Z": Zp[k * ROWS_PER_CORE : (k + 1) * ROWS_PER_CORE]}
        for k in range(NCORES)
    ]

    res = run_bass_kernel_spmd(nc, in_maps, core_ids=list(range(NCORES)))
    _cache["last_results"] = res

    gj = host_extra.copy()
    for k, r in enumerate(res.results):
        # [128, 160] -> bands summed -> [32, 5, 32] per-chunk partials
        g = r["grams"].astype(np.float64).reshape(4, D, CHUNKS_PER_CORE, D).sum(axis=0)
        for c in range(CHUNKS_PER_CORE):
            gj[(k * CHUNKS_PER_CORE + c) // CHUNKS_PER_CLASS] += g[:, c, :]

    g_all = gj.sum(axis=0)
    tr_pi = counts.astype(np.float64)

    nf, df = float(N), float(D)
    eye = np.eye(D)
    loss_r = 0.5 * np.linalg.slogdet(eye + (df / (nf * EPS)) * g_all)[1]
    loss_rc = 0.0
    for j in range(C):
        ld = np.linalg.slogdet(eye + (df / (tr_pi[j] * EPS)) * gj[j])[1]
        loss_rc += (tr_pi[j] / (2.0 * nf)) * ld
    loss_obj = loss_r - loss_rc
    return np.asarray([-loss_obj, loss_r, loss_rc], dtype=np.float32)
